# revision 3
# baseline (speedup 1.0000x reference)
"""GQA causal self-attention (B=2, T=2048, C=2048, 16 Q heads / 4 KV heads,
head_dim=128) on 8 TRN2 NeuronCores.

Sharding: core = (batch b, kv-group g) for b in {0,1}, g in {0..3}.
Each core computes its batch's 4 Q heads that share KV head g, plus the
partial out-projection over those heads' rows of W_out. Host sums the 4
partials per batch and adds b_out.

Device layout choices (all feature-major, "T" on the free axis):
  - QKV projection for q/k chunks runs in fp8e4 with DoubleRow perf mode
    (contraction pairs of 128-row blocks -> 8 matmuls instead of 16).
    W_qkv q/k columns are pre-scaled x16 on the host so fp8 keeps full
    relative precision; the extra 256x on the scores folds into the exp
    scale. The v chunk stays bf16 (value-path precision).
  - qT/kT [d=128 part, t free]  -> scores S^T[j,i] = kT_tile.T @ qT_slice
  - softmax over j (= partition axis of S^T): exp on ACT (scale fused),
    causal mask via gpsimd.affine_select; the denominator accumulates on
    the Pool engine (tensor_copy/tensor_tensor adds of P tiles) and
    reduces across partitions with gpsimd.partition_all_reduce — no PE
    cycles spent on the ones-vector matmul.
  - y^T[d, i] = v_tile.T-free accumulation: lhsT = v[t,d] tiles, rhs = P^T.
  - out[t, e] = y^T as lhsT directly against W_out rows.
Matmul operands bf16 except the q/k QKV projection (fp8 DoubleRow);
fp32 PSUM accumulation; everything else fp32.
"""

import sys

if "/opt/trn_rl_repo" not in sys.path:
    sys.path.insert(0, "/opt/trn_rl_repo")

import numpy as np
import ml_dtypes

BF16 = ml_dtypes.bfloat16
F8E4 = ml_dtypes.float8_e4m3

B = 2
T = 2048
C = 2048
NH = 16
NKV = 4
D = 128
GQ = NH // NKV  # 4 q heads per kv head
N_CORES = 8
CC = C // 128  # 16 contraction chunks
TS = T // 512  # 4 t-slices
TT = T // 128  # 16 t-tiles
NQK = GQ + 1  # q0..q3, k chunks (fp8 path)
WSCALE = 16.0  # host pre-scale on W_qkv q/k cols + their biases

_CACHED = {}


def _build_bass(reps=1):
    import concourse.bass as bass
    import concourse.bacc as bacc
    import concourse.tile as tile
    import concourse.mybir as mybir

    bf = mybir.dt.bfloat16
    f8 = mybir.dt.float8e4
    f32 = mybir.dt.float32
    Exp = mybir.ActivationFunctionType.Exp

    nc = bacc.Bacc(None, target_bir_lowering=False)

    # DRAM inputs (host pre-laid-out, see kernel())
    xt8_d = nc.dram_tensor("xt8", [128, CC, T], f8, kind="ExternalInput")
    xtv_d = nc.dram_tensor("xtv", [128, CC, T], bf, kind="ExternalInput")
    wqk8_d = nc.dram_tensor("wqk8", [128, CC, NQK * 128], f8, kind="ExternalInput")
    wv_d = nc.dram_tensor("wv", [128, CC, 128], bf, kind="ExternalInput")
    bqkv_d = nc.dram_tensor("bqkv", [128, NQK + 1], f32, kind="ExternalInput")
    cos_d = nc.dram_tensor("cosT", [128, T], f32, kind="ExternalInput")
    sin_d = nc.dram_tensor("sinT", [128, T], f32, kind="ExternalInput")
    swap_d = nc.dram_tensor("swp", [128, 128], bf, kind="ExternalInput")
    iden_d = nc.dram_tensor("idn", [128, 128], bf, kind="ExternalInput")
    wout_d = nc.dram_tensor("wout", [128, GQ, C], bf, kind="ExternalInput")
    out_d = nc.dram_tensor("out", [T, C], f32, kind="ExternalOutput")

    with tile.TileContext(nc) as tc:
        with (
            tc.tile_pool(name="persist", bufs=1) as pers,
            tc.tile_pool(name="xt", bufs=2) as xtp,
            tc.tile_pool(name="stage", bufs=3) as stg,
            tc.tile_pool(name="ptile", bufs=6) as ptp,
            tc.tile_pool(name="small", bufs=4) as smp,
            tc.tile_pool(name="dacc", bufs=3) as dap,
            tc.tile_pool(name="osb", bufs=3) as osp,
            tc.tile_pool(name="ps_qkv", bufs=2, space="PSUM") as ppq,
            tc.tile_pool(name="ps_sc", bufs=2, space="PSUM") as pps,
            tc.tile_pool(name="ps_y", bufs=2, space="PSUM") as ppy,
        ):
            import contextlib
            loop_cm = tc.For_i(0, reps, 1) if reps > 1 else contextlib.nullcontext()
            with loop_cm:
                _body(nc, tc, mybir, bf, f8, f32, Exp,
                      pers, xtp, stg, ptp, smp, dap, osp, ppq, pps, ppy,
                      xt8_d, xtv_d, wqk8_d, wv_d, bqkv_d, cos_d, sin_d,
                      swap_d, iden_d, wout_d, out_d)
    nc.compile()
    return nc


def _body(nc, tc, mybir, bf, f8, f32, Exp,
          pers, xtp, stg, ptp, smp, dap, osp, ppq, pps, ppy,
          xt8_d, xtv_d, wqk8_d, wv_d, bqkv_d, cos_d, sin_d,
          swap_d, iden_d, wout_d, out_d):
            from concourse import bass_isa
            DR = mybir.MatmulPerfMode.DoubleRow

            # ---- persistent loads ----
            wqk8_sb = pers.tile([128, CC, NQK * 128], f8)
            nc.sync.dma_start(wqk8_sb[:], wqk8_d[:])
            wv_sb = pers.tile([128, CC, 128], bf)
            nc.sync.dma_start(wv_sb[:], wv_d[:])
            xt8_0 = xtp.tile([128, CC, 512], f8, tag="xt8")
            nc.sync.dma_start(xt8_0[:], xt8_d[:, :, 0:512])
            xtv_0 = xtp.tile([128, CC, 512], bf, tag="xtv")
            for xc in range(4):
                nc.sync.dma_start(xtv_0[:, xc * 4 : (xc + 1) * 4, :],
                                  xtv_d[:, xc * 4 : (xc + 1) * 4, 0:512])
            bq_sb = pers.tile([128, NQK + 1], f32)
            nc.sync.dma_start(bq_sb[:], bqkv_d[:])
            swap_sb = pers.tile([128, 128], bf)
            nc.sync.dma_start(swap_sb[:], swap_d[:])
            iden_sb = pers.tile([128, 128], bf)
            nc.sync.dma_start(iden_sb[:], iden_d[:])
            cos_sb = pers.tile([128, T], f32)
            nc.sync.dma_start(cos_sb[:], cos_d[:])
            sin_sb = pers.tile([128, T], f32)
            nc.sync.dma_start(sin_sb[:], sin_d[:])
            wout_sb = pers.tile([128, GQ, C], bf)
            nc.sync.dma_start(wout_sb[:], wout_d[:])

            # persistent activations
            qk_sb = pers.tile([128, NQK, T], bf)  # rotated q0..q3, k
            v_sb = pers.tile([128, TT, 128], bf)  # v in [t-part, d] tiles
            y_sb = pers.tile([128, GQ, T], bf)  # y^T per head

            mul = mybir.AluOpType.mult
            add = mybir.AluOpType.add

            # ---- phase 1: QKV + RoPE + v transpose ----
            for ts in range(TS):
                tsl = slice(ts * 512, (ts + 1) * 512)
                if ts == 0:
                    xt8 = xt8_0
                    xtv = xtv_0
                else:
                    xt8 = xtp.tile([128, CC, 512], f8, tag="xt8")
                    nc.sync.dma_start(xt8[:], xt8_d[:, :, tsl])
                    xtv = xtp.tile([128, CC, 512], bf, tag="xtv")
                    nc.sync.dma_start(xtv[:], xtv_d[:, :, tsl])
                # q0..q3, k in fp8 DoubleRow (8 contraction-pair matmuls)
                for f in range(NQK):
                    ps = ppq.tile([128, 512], f32, tag="qkvps")
                    for cp in range(CC // 2):
                        nc.tensor.matmul(
                            ps[:],
                            wqk8_sb[:, 2 * cp : 2 * cp + 2,
                                    f * 128 : (f + 1) * 128],
                            xt8[:, 2 * cp : 2 * cp + 2, :],
                            start=(cp == 0),
                            stop=(cp == CC // 2 - 1),
                            perf_mode=DR,
                        )
                    # bias add (also PSUM->SBUF move), bf16 out
                    raw = stg.tile([128, 512], bf, tag="raw")
                    nc.vector.tensor_tensor(
                        raw[:], ps[:], bq_sb[:, f : f + 1].to_broadcast((128, 512)),
                        add,
                    )
                    # rope: rot = raw*cos + swap(raw)*sinsign
                    psw = pps.tile([128, 512], f32, tag="sps")
                    nc.tensor.matmul(psw[:], swap_sb[:], raw[:], start=True,
                                     stop=True)
                    tmp = stg.tile([128, 512], bf, tag="ropetmp")
                    nc.vector.tensor_tensor(tmp[:], psw[:], sin_sb[:, tsl], mul)
                    nc.vector.tensor_tensor(
                        qk_sb[:, f, tsl], raw[:], cos_sb[:, tsl], mul
                    )
                    nc.vector.tensor_tensor(
                        qk_sb[:, f, tsl], qk_sb[:, f, tsl], tmp[:], add
                    )
                # v in bf16
                ps = ppq.tile([128, 512], f32, tag="qkvps")
                for cc in range(CC):
                    nc.tensor.matmul(
                        ps[:],
                        wv_sb[:, cc, :],
                        xtv[:, cc, :],
                        start=(cc == 0),
                        stop=(cc == CC - 1),
                    )
                raw = stg.tile([128, 512], bf, tag="raw")
                nc.vector.tensor_tensor(
                    raw[:], ps[:], bq_sb[:, NQK : NQK + 1].to_broadcast((128, 512)),
                    add,
                )
                # v: transpose [d, t] -> [t, d] via PE
                for k in range(4):
                    pst = pps.tile([128, 128], bf, tag="sps")
                    nc.tensor.transpose(
                        pst[:], raw[:, k * 128 : (k + 1) * 128], iden_sb[:]
                    )
                    nc.any.tensor_copy(v_sb[:, ts * 4 + k, :], pst[:])

            # ---- phase 2+3: per i-slice: attention (4 heads) then out-proj ----
            for s in range(TS):
                isl = slice(s * 512, (s + 1) * 512)
                njt = 4 * (s + 1)
                for h in range(GQ):
                    psy = ppy.tile([128, 512], f32, tag="yps")
                    dacc = dap.tile([128, 512], f32, tag="dacc")
                    for jt in range(njt):
                        # columns i < 128*jt are fully masked: skip them
                        off = max(0, 128 * jt - 512 * s)
                        pss = pps.tile([128, 512], f32, tag="sps")
                        nc.tensor.matmul(
                            pss[:, off:512],
                            qk_sb[:, GQ, jt * 128 : (jt + 1) * 128],
                            qk_sb[:, h, s * 512 + off : (s + 1) * 512],
                            start=True,
                            stop=True,
                        )
                        P = ptp.tile([128, 512], bf, tag="P")
                        nc.scalar.activation(
                            P[:, off:512], pss[:, off:512], Exp,
                            scale=1.0 / (128.0 * WSCALE * WSCALE),
                        )
                        if jt >= 4 * s:
                            # triangular block: keep where p <= y (y rel. to off)
                            nc.gpsimd.affine_select(
                                out=P[:, off : off + 128],
                                in_=P[:, off : off + 128],
                                pattern=[[1, 128]],
                                compare_op=mybir.AluOpType.is_ge,
                                fill=0.0,
                                base=0,
                                channel_multiplier=-1,
                            )
                        nc.tensor.matmul(
                            psy[:, off:512],
                            v_sb[:, jt, :],
                            P[:, off:512],
                            start=(jt == 0),
                            stop=(jt == njt - 1),
                        )
                        # denominator accumulation on Pool (f32)
                        if jt == 0:
                            nc.gpsimd.tensor_copy(dacc[:], P[:])
                        else:
                            nc.gpsimd.tensor_tensor(
                                dacc[:, off:512], dacc[:, off:512], P[:, off:512],
                                add,
                            )
                    den_b = smp.tile([128, 512], f32, tag="denb")
                    nc.gpsimd.partition_all_reduce(
                        den_b[:], dacc[:], 128, bass_isa.ReduceOp.add
                    )
                    rdb = smp.tile([128, 512], f32, tag="rdb")
                    nc.vector.reciprocal(rdb[:], den_b[:])
                    nc.vector.tensor_tensor(y_sb[:, h, isl], psy[:], rdb[:], mul)

                for tt in range(4 * s, 4 * s + 4):
                    o_sb = osp.tile([128, C], f32, tag="osb")
                    for es in range(4):
                        pso = ppy.tile([128, 512], f32, tag="yps")
                        for h in range(GQ):
                            nc.tensor.matmul(
                                pso[:],
                                y_sb[:, h, tt * 128 : (tt + 1) * 128],
                                wout_sb[:, h, es * 512 : (es + 1) * 512],
                                start=(h == 0),
                                stop=(h == GQ - 1),
                            )
                        if es % 2 == 0:
                            nc.vector.tensor_copy(
                                o_sb[:, es * 512 : (es + 1) * 512], pso[:]
                            )
                        else:
                            nc.scalar.copy(
                                o_sb[:, es * 512 : (es + 1) * 512], pso[:]
                            )
                    nc.sync.dma_start(out_d[tt * 128 : (tt + 1) * 128, :], o_sb[:])


def _host_prep(x, rope_cache, W_qkv, b_qkv, W_out):
    """Build the 8 per-core input dicts."""
    q_dim = NH * D  # 2048
    kv_dim = NKV * D  # 512

    # rope tables in [d, t] layout
    sin = rope_cache[:, 0::2].astype(np.float32)  # [T, 64]
    cos = rope_cache[:, 1::2].astype(np.float32)
    cos2T = np.empty((128, T), np.float32)
    sinsT = np.empty((128, T), np.float32)
    cos2T[0::2] = cos.T
    cos2T[1::2] = cos.T
    sinsT[0::2] = -sin.T
    sinsT[1::2] = sin.T

    swap = np.zeros((128, 128), BF16)
    idx = np.arange(128)
    swap[idx, idx ^ 1] = 1
    iden = np.eye(128, dtype=BF16)

    in_maps = []
    for b in range(B):
        xT = np.ascontiguousarray(x[b].T)  # [C, T] f32
        xT = xT.reshape(CC, 128, T).transpose(1, 0, 2)  # [128, CC, T]
        xT8 = np.ascontiguousarray(xT).astype(F8E4)
        xTv = np.ascontiguousarray(xT).astype(BF16)
        for g in range(NKV):
            qk_cols = np.concatenate(
                [
                    np.arange(4 * g * D, (4 * g + 4) * D),  # 4 q heads
                    np.arange(q_dim + g * D, q_dim + (g + 1) * D),  # k head
                ]
            )
            v_cols = np.arange(q_dim + kv_dim + g * D, q_dim + kv_dim + (g + 1) * D)
            wqk = (W_qkv[:, qk_cols].astype(np.float32) * WSCALE).astype(F8E4)
            wqk = np.ascontiguousarray(
                wqk.reshape(CC, 128, NQK * 128).transpose(1, 0, 2)
            )  # [128, CC, 640]
            wv = W_qkv[:, v_cols].astype(BF16)
            wv = np.ascontiguousarray(
                wv.reshape(CC, 128, 128).transpose(1, 0, 2)
            )  # [128, CC, 128]
            bq = np.concatenate(
                [b_qkv[qk_cols].astype(np.float32) * WSCALE,
                 b_qkv[v_cols].astype(np.float32)]
            ).reshape(NQK + 1, 128).T  # [128, NQK+1]
            bq = np.ascontiguousarray(bq)
            wo = W_out[4 * g * D : (4 * g + 4) * D, :].astype(BF16)  # [512, C]
            wo = np.ascontiguousarray(
                wo.reshape(GQ, 128, C).transpose(1, 0, 2)
            )  # [128, GQ, C]
            in_maps.append(
                {
                    "xt8": xT8,
                    "xtv": xTv,
                    "wqk8": wqk,
                    "wv": wv,
                    "bqkv": bq,
                    "cosT": cos2T,
                    "sinT": sinsT,
                    "swp": swap,
                    "idn": iden,
                    "wout": wo,
                }
            )
    return in_maps


def kernel(x, rope_cache, W_qkv, b_qkv, W_out, b_out, _trace=False):
    from concourse.bass_utils import run_bass_kernel_spmd

    if "nc" not in _CACHED:
        _CACHED["nc"] = _build_bass()
    nc = _CACHED["nc"]

    in_maps = _host_prep(
        np.asarray(x), np.asarray(rope_cache), np.asarray(W_qkv),
        np.asarray(b_qkv), np.asarray(W_out),
    )
    res = run_bass_kernel_spmd(nc, in_maps, core_ids=list(range(N_CORES)), trace=_trace)
    _CACHED["last_result"] = res

    out = np.zeros((B, T, C), np.float32)
    for b in range(B):
        acc = res.results[b * NKV]["out"].astype(np.float32)
        for g in range(1, NKV):
            acc = acc + res.results[b * NKV + g]["out"]
        out[b] = acc + np.asarray(b_out)[None, :]
    return out


# revision 22
# speedup vs baseline: 4.0711x; 4.0711x over previous
"""GQA causal self-attention (B=2, T=2048, C=2048, 16 Q heads / 4 KV heads,
head_dim=128) on 8 TRN2 NeuronCores.

Sharding: core = (batch b, kv-group g) for b in {0,1}, g in {0..3}.
Each core computes its batch's 4 Q heads that share KV head g, plus the
partial out-projection over those heads' rows of W_out. Host sums the 4
partials per batch and adds b_out.

Device layout choices (all feature-major, "T" on the free axis):
  - QKV projection for q/k chunks runs in fp8e4 with DoubleRow perf mode
    (contraction pairs of 128-row blocks -> 8 matmuls instead of 16).
    W_qkv q/k columns are pre-scaled x16 on the host so fp8 keeps full
    relative precision; the extra 256x on the scores folds into the exp
    scale. The v chunk stays bf16 (value-path precision).
  - qT/kT [d=128 part, t free]  -> scores S^T[j,i] = kT_tile.T @ qT_slice
  - softmax over j (= partition axis of S^T): exp on ACT (scale fused),
    causal mask via gpsimd.affine_select; the denominator accumulates on
    the Pool engine (tensor_copy/tensor_tensor adds of P tiles) and
    reduces across partitions with gpsimd.partition_all_reduce — no PE
    cycles spent on the ones-vector matmul.
  - y^T[d, i] = v_tile.T-free accumulation: lhsT = v[t,d] tiles, rhs = P^T.
  - out[t, e] = y^T as lhsT directly against W_out rows.
Matmul operands bf16 except the q/k QKV projection (fp8 DoubleRow);
fp32 PSUM accumulation; everything else fp32.
"""

import sys

if "/opt/trn_rl_repo" not in sys.path:
    sys.path.insert(0, "/opt/trn_rl_repo")

import numpy as np
import ml_dtypes

BF16 = ml_dtypes.bfloat16
F8E4 = ml_dtypes.float8_e4m3

B = 2
T = 2048
C = 2048
NH = 16
NKV = 4
D = 128
GQ = NH // NKV  # 4 q heads per kv head
N_CORES = 8
CC = C // 128  # 16 contraction chunks
TS = T // 512  # 4 t-slices
TT = T // 128  # 16 t-tiles
NQK = GQ + 1  # q0..q3, k chunks (fp8 path)
WSCALE = 16.0  # host pre-scale on W_qkv q/k cols + their biases

# feature flags (bisectable)
USE_DR = True     # fp8 DoubleRow for q/k QKV projection
POOL_DEN = False  # softmax denominator on Pool engine instead of PE matmul
OUT_BF16 = True   # write out-proj partials as bf16 (halves output DMA)

_CACHED = {}


def _build_bass(reps=1, use_dr=None, pool_den=None, out_bf16=None):
    import concourse.bass as bass
    import concourse.bacc as bacc
    import concourse.tile as tile
    import concourse.mybir as mybir

    if use_dr is None:
        use_dr = USE_DR
    if pool_den is None:
        pool_den = POOL_DEN
    if out_bf16 is None:
        out_bf16 = OUT_BF16

    bf = mybir.dt.bfloat16
    f8 = mybir.dt.float8e4
    f32 = mybir.dt.float32
    Exp = mybir.ActivationFunctionType.Exp

    nc = bacc.Bacc(None, target_bir_lowering=False)

    # DRAM inputs (host pre-laid-out, see kernel())
    xt8_d = nc.dram_tensor("xt8", [128, CC, T], f8, kind="ExternalInput")
    xtv_d = nc.dram_tensor("xtv", [128, CC, T], bf, kind="ExternalInput")
    wqkb_d = nc.dram_tensor("wqkb", [128, CC, NQK * 128], bf, kind="ExternalInput")
    wqk8_d = nc.dram_tensor("wqk8", [128, CC, NQK * 128], f8, kind="ExternalInput")
    wv_d = nc.dram_tensor("wv", [128, CC, 128], bf, kind="ExternalInput")
    bqkv_d = nc.dram_tensor("bqkv", [128, NQK + 1], f32, kind="ExternalInput")
    cos_d = nc.dram_tensor("cosT", [128, T], f32, kind="ExternalInput")
    sin_d = nc.dram_tensor("sinT", [128, T], f32, kind="ExternalInput")
    swap_d = nc.dram_tensor("swp", [128, 128], bf, kind="ExternalInput")
    iden_d = nc.dram_tensor("idn", [128, 128], bf, kind="ExternalInput")
    wout_d = nc.dram_tensor("wout", [128, GQ, C], bf, kind="ExternalInput")
    out_d = nc.dram_tensor("out", [T, C], bf if out_bf16 else f32,
                           kind="ExternalOutput")

    with tile.TileContext(nc) as tc:
        with (
            tc.tile_pool(name="persist", bufs=1) as pers,
            tc.tile_pool(name="xt", bufs=2) as xtp,
            tc.tile_pool(name="stage", bufs=3) as stg,
            tc.tile_pool(name="ptile", bufs=6) as ptp,
            tc.tile_pool(name="small", bufs=4) as smp,
            tc.tile_pool(name="dacc", bufs=3) as dap,
            tc.tile_pool(name="osb", bufs=3) as osp,
            tc.tile_pool(name="ps_qkv", bufs=2, space="PSUM") as ppq,
            tc.tile_pool(name="ps_sc", bufs=2, space="PSUM") as pps,
            tc.tile_pool(name="ps_y", bufs=2, space="PSUM") as ppy,
            tc.tile_pool(name="ps_d", bufs=2, space="PSUM") as ppd,
        ):
            import contextlib
            loop_cm = tc.For_i(0, reps, 1) if reps > 1 else contextlib.nullcontext()
            with loop_cm:
                _body(nc, tc, mybir, bf, f8, f32, Exp,
                      pers, xtp, stg, ptp, smp, dap, osp, ppq, pps, ppy, ppd,
                      xt8_d, xtv_d, wqkb_d, wqk8_d, wv_d, bqkv_d, cos_d, sin_d,
                      swap_d, iden_d, wout_d, out_d, use_dr, pool_den, out_bf16)
    nc.compile()
    return nc


def _body(nc, tc, mybir, bf, f8, f32, Exp,
          pers, xtp, stg, ptp, smp, dap, osp, ppq, pps, ppy, ppd,
          xt8_d, xtv_d, wqkb_d, wqk8_d, wv_d, bqkv_d, cos_d, sin_d,
          swap_d, iden_d, wout_d, out_d, use_dr, pool_den, out_bf16):
            from concourse import bass_isa
            DR = mybir.MatmulPerfMode.DoubleRow

            # ---- persistent loads ----
            if use_dr:
                wqk8_sb = pers.tile([128, CC, NQK * 128], f8)
                nc.sync.dma_start(wqk8_sb[:], wqk8_d[:])
            else:
                wqkb_sb = pers.tile([128, CC, NQK * 128], bf)
                nc.sync.dma_start(wqkb_sb[:], wqkb_d[:])
            wv_sb = pers.tile([128, CC, 128], bf)
            nc.sync.dma_start(wv_sb[:], wv_d[:])
            if use_dr:
                xt8_0 = xtp.tile([128, CC, 512], f8, tag="xt8")
                nc.sync.dma_start(xt8_0[:], xt8_d[:, :, 0:512])
            xtv_0 = xtp.tile([128, CC, 512], bf, tag="xtv")
            for xc in range(4):
                nc.sync.dma_start(xtv_0[:, xc * 4 : (xc + 1) * 4, :],
                                  xtv_d[:, xc * 4 : (xc + 1) * 4, 0:512])
            bq_sb = pers.tile([128, NQK + 1], f32)
            nc.sync.dma_start(bq_sb[:], bqkv_d[:])
            swap_sb = pers.tile([128, 128], bf)
            nc.sync.dma_start(swap_sb[:], swap_d[:])
            iden_sb = pers.tile([128, 128], bf)
            nc.sync.dma_start(iden_sb[:], iden_d[:])
            cos_sb = pers.tile([128, T], f32)
            nc.sync.dma_start(cos_sb[:], cos_d[:])
            sin_sb = pers.tile([128, T], f32)
            nc.sync.dma_start(sin_sb[:], sin_d[:])
            wout_sb = pers.tile([128, GQ, C], bf)
            nc.sync.dma_start(wout_sb[:], wout_d[:])

            # persistent activations
            qk_sb = pers.tile([128, NQK, T], bf)  # rotated q0..q3, k
            v_sb = pers.tile([128, TT, 128], bf)  # v in [t-part, d] tiles
            y_sb = pers.tile([128, GQ, T], bf)  # y^T per head
            if not pool_den:
                ones_sb = pers.tile([128, 1], bf)
                nc.vector.memset(ones_sb[:], 1.0)

            mul = mybir.AluOpType.mult
            add = mybir.AluOpType.add

            # ---- phase 1: QKV + RoPE + v transpose ----
            for ts in range(TS):
                tsl = slice(ts * 512, (ts + 1) * 512)
                if ts == 0:
                    xt8 = xt8_0 if use_dr else None
                    xtv = xtv_0
                else:
                    if use_dr:
                        xt8 = xtp.tile([128, CC, 512], f8, tag="xt8")
                        nc.sync.dma_start(xt8[:], xt8_d[:, :, tsl])
                    xtv = xtp.tile([128, CC, 512], bf, tag="xtv")
                    nc.sync.dma_start(xtv[:], xtv_d[:, :, tsl])
                # q0..q3, k in fp8 DoubleRow (8 contraction-pair matmuls)
                for f in range(NQK):
                    ps = ppq.tile([128, 512], f32, tag="qkvps")
                    if use_dr:
                        for cp in range(CC // 2):
                            nc.tensor.matmul(
                                ps[:],
                                wqk8_sb[:, 2 * cp : 2 * cp + 2,
                                        f * 128 : (f + 1) * 128],
                                xt8[:, 2 * cp : 2 * cp + 2, :],
                                start=(cp == 0),
                                stop=(cp == CC // 2 - 1),
                                perf_mode=DR,
                            )
                    else:
                        for cc in range(CC):
                            nc.tensor.matmul(
                                ps[:],
                                wqkb_sb[:, cc, f * 128 : (f + 1) * 128],
                                xtv[:, cc, :],
                                start=(cc == 0),
                                stop=(cc == CC - 1),
                            )
                    # bias add (also PSUM->SBUF move), bf16 out
                    raw = stg.tile([128, 512], bf, tag="raw")
                    nc.vector.tensor_tensor(
                        raw[:], ps[:], bq_sb[:, f : f + 1].to_broadcast((128, 512)),
                        add,
                    )
                    # rope: rot = raw*cos + swap(raw)*sinsign
                    psw = pps.tile([128, 512], f32, tag="sps")
                    nc.tensor.matmul(psw[:], swap_sb[:], raw[:], start=True,
                                     stop=True)
                    tmp = stg.tile([128, 512], bf, tag="ropetmp")
                    nc.vector.tensor_tensor(tmp[:], psw[:], sin_sb[:, tsl], mul)
                    nc.vector.tensor_tensor(
                        qk_sb[:, f, tsl], raw[:], cos_sb[:, tsl], mul
                    )
                    nc.vector.tensor_tensor(
                        qk_sb[:, f, tsl], qk_sb[:, f, tsl], tmp[:], add
                    )
                # v in bf16
                ps = ppq.tile([128, 512], f32, tag="qkvps")
                for cc in range(CC):
                    nc.tensor.matmul(
                        ps[:],
                        wv_sb[:, cc, :],
                        xtv[:, cc, :],
                        start=(cc == 0),
                        stop=(cc == CC - 1),
                    )
                raw = stg.tile([128, 512], bf, tag="raw")
                nc.vector.tensor_tensor(
                    raw[:], ps[:], bq_sb[:, NQK : NQK + 1].to_broadcast((128, 512)),
                    add,
                )
                # v: transpose [d, t] -> [t, d] via PE
                for k in range(4):
                    pst = pps.tile([128, 128], bf, tag="sps")
                    nc.tensor.transpose(
                        pst[:], raw[:, k * 128 : (k + 1) * 128], iden_sb[:]
                    )
                    nc.any.tensor_copy(v_sb[:, ts * 4 + k, :], pst[:])

            # ---- phase 2+3: per i-slice: attention (4 heads) then out-proj ----
            for s in range(TS):
                isl = slice(s * 512, (s + 1) * 512)
                njt = 4 * (s + 1)
                for h in range(GQ):
                    psy = ppy.tile([128, 512], f32, tag="yps")
                    if pool_den:
                        dacc = dap.tile([128, 512], f32, tag="dacc")
                    else:
                        psd = ppd.tile([1, 512], f32, tag="dps")
                    for jt in range(njt):
                        # columns i < 128*jt are fully masked: skip them
                        off = max(0, 128 * jt - 512 * s)
                        pss = pps.tile([128, 512], f32, tag="sps")
                        nc.tensor.matmul(
                            pss[:, off:512],
                            qk_sb[:, GQ, jt * 128 : (jt + 1) * 128],
                            qk_sb[:, h, s * 512 + off : (s + 1) * 512],
                            start=True,
                            stop=True,
                        )
                        P = ptp.tile([128, 512], bf, tag="P")
                        nc.scalar.activation(
                            P[:, off:512], pss[:, off:512], Exp,
                            scale=1.0 / (128.0 * WSCALE * WSCALE),
                        )
                        if jt >= 4 * s:
                            # triangular block: keep where p <= y (y rel. to off)
                            nc.gpsimd.affine_select(
                                out=P[:, off : off + 128],
                                in_=P[:, off : off + 128],
                                pattern=[[1, 128]],
                                compare_op=mybir.AluOpType.is_ge,
                                fill=0.0,
                                base=0,
                                channel_multiplier=-1,
                            )
                        nc.tensor.matmul(
                            psy[:, off:512],
                            v_sb[:, jt, :],
                            P[:, off:512],
                            start=(jt == 0),
                            stop=(jt == njt - 1),
                        )
                        if pool_den:
                            # denominator accumulation on Pool (f32)
                            if jt == 0:
                                nc.gpsimd.tensor_copy(dacc[:], P[:])
                            else:
                                nc.gpsimd.tensor_tensor(
                                    dacc[:, off:512], dacc[:, off:512],
                                    P[:, off:512], add,
                                )
                        else:
                            nc.tensor.matmul(
                                psd[:, off:512],
                                ones_sb[:],
                                P[:, off:512],
                                start=(jt == 0),
                                stop=(jt == njt - 1),
                            )
                    if pool_den:
                        den_b = smp.tile([128, 512], f32, tag="denb")
                        nc.gpsimd.partition_all_reduce(
                            den_b[:], dacc[:], 128, bass_isa.ReduceOp.add
                        )
                        rdb = smp.tile([128, 512], f32, tag="rdb")
                        nc.vector.reciprocal(rdb[:], den_b[:])
                    else:
                        rden = smp.tile([1, 512], f32, tag="rden")
                        nc.vector.reciprocal(rden[:], psd[:])
                        rdb = smp.tile([128, 512], f32, tag="rdb")
                        nc.gpsimd.partition_broadcast(rdb[:], rden[:])
                    nc.vector.tensor_tensor(y_sb[:, h, isl], psy[:], rdb[:], mul)

                for tt in range(4 * s, 4 * s + 4):
                    o_sb = osp.tile([128, C], bf if out_bf16 else f32, tag="osb")
                    for es in range(4):
                        pso = ppy.tile([128, 512], f32, tag="yps")
                        for h in range(GQ):
                            nc.tensor.matmul(
                                pso[:],
                                y_sb[:, h, tt * 128 : (tt + 1) * 128],
                                wout_sb[:, h, es * 512 : (es + 1) * 512],
                                start=(h == 0),
                                stop=(h == GQ - 1),
                            )
                        if es % 2 == 0:
                            nc.vector.tensor_copy(
                                o_sb[:, es * 512 : (es + 1) * 512], pso[:]
                            )
                        else:
                            nc.scalar.copy(
                                o_sb[:, es * 512 : (es + 1) * 512], pso[:]
                            )
                    nc.sync.dma_start(out_d[tt * 128 : (tt + 1) * 128, :], o_sb[:])


def _host_prep(x, rope_cache, W_qkv, b_qkv, W_out):
    """Build the 8 per-core input dicts."""
    q_dim = NH * D  # 2048
    kv_dim = NKV * D  # 512

    # rope tables in [d, t] layout
    sin = rope_cache[:, 0::2].astype(np.float32)  # [T, 64]
    cos = rope_cache[:, 1::2].astype(np.float32)
    cos2T = np.empty((128, T), np.float32)
    sinsT = np.empty((128, T), np.float32)
    cos2T[0::2] = cos.T
    cos2T[1::2] = cos.T
    sinsT[0::2] = -sin.T
    sinsT[1::2] = sin.T

    swap = np.zeros((128, 128), BF16)
    idx = np.arange(128)
    swap[idx, idx ^ 1] = 1
    iden = np.eye(128, dtype=BF16)

    in_maps = []
    for b in range(B):
        xT = np.ascontiguousarray(x[b].T)  # [C, T] f32
        xT = xT.reshape(CC, 128, T).transpose(1, 0, 2)  # [128, CC, T]
        xT8 = np.ascontiguousarray(xT).astype(F8E4)
        xTv = np.ascontiguousarray(xT).astype(BF16)
        for g in range(NKV):
            qk_cols = np.concatenate(
                [
                    np.arange(4 * g * D, (4 * g + 4) * D),  # 4 q heads
                    np.arange(q_dim + g * D, q_dim + (g + 1) * D),  # k head
                ]
            )
            v_cols = np.arange(q_dim + kv_dim + g * D, q_dim + kv_dim + (g + 1) * D)
            wqk_scaled = W_qkv[:, qk_cols].astype(np.float32) * WSCALE
            wqk = np.ascontiguousarray(
                wqk_scaled.astype(F8E4)
                .reshape(CC, 128, NQK * 128).transpose(1, 0, 2)
            )  # [128, CC, 640]
            wqkb = np.ascontiguousarray(
                wqk_scaled.astype(BF16)
                .reshape(CC, 128, NQK * 128).transpose(1, 0, 2)
            )  # [128, CC, 640] bf16 (non-DR fallback)
            wv = W_qkv[:, v_cols].astype(BF16)
            wv = np.ascontiguousarray(
                wv.reshape(CC, 128, 128).transpose(1, 0, 2)
            )  # [128, CC, 128]
            bq = np.concatenate(
                [b_qkv[qk_cols].astype(np.float32) * WSCALE,
                 b_qkv[v_cols].astype(np.float32)]
            ).reshape(NQK + 1, 128).T  # [128, NQK+1]
            bq = np.ascontiguousarray(bq)
            wo = W_out[4 * g * D : (4 * g + 4) * D, :].astype(BF16)  # [512, C]
            wo = np.ascontiguousarray(
                wo.reshape(GQ, 128, C).transpose(1, 0, 2)
            )  # [128, GQ, C]
            in_maps.append(
                {
                    "xt8": xT8,
                    "xtv": xTv,
                    "wqk8": wqk,
                    "wqkb": wqkb,
                    "wv": wv,
                    "bqkv": bq,
                    "cosT": cos2T,
                    "sinT": sinsT,
                    "swp": swap,
                    "idn": iden,
                    "wout": wo,
                }
            )
    return in_maps


def kernel(x, rope_cache, W_qkv, b_qkv, W_out, b_out, _trace=False):
    from concourse.bass_utils import run_bass_kernel_spmd

    if "nc" not in _CACHED:
        _CACHED["nc"] = _build_bass()
    nc = _CACHED["nc"]

    in_maps = _host_prep(
        np.asarray(x), np.asarray(rope_cache), np.asarray(W_qkv),
        np.asarray(b_qkv), np.asarray(W_out),
    )
    res = run_bass_kernel_spmd(nc, in_maps, core_ids=list(range(N_CORES)), trace=_trace)
    _CACHED["last_result"] = res

    out = np.zeros((B, T, C), np.float32)
    for b in range(B):
        acc = res.results[b * NKV]["out"].astype(np.float32)
        for g in range(1, NKV):
            acc = acc + res.results[b * NKV + g]["out"]
        out[b] = acc + np.asarray(b_out)[None, :]
    return out
